# revision 2
# baseline (speedup 1.0000x reference)
"""Trainium2 Bass kernel for nn_BPDecoder: logits = 1 - exp(-exp(sum_i R_i*||Z_i||^2)).

v2 strategy (8-core SPMD, row-sharded, fp8 end-to-end):
  - Error budget: logits tolerance 2e-2 allows ~30% relative error on the
    scalar s, so fp8 e4m3 is used for Z (host scales by 512), R, AND the
    squares; the result is divided by 512^2 on host.  Measured ~2e-3.
  - Row r of a core maps to (partition, tile, q) = (r//496, (r%496)//16,
    r%16): the per-core wire buffer is literally Zfp8.reshape(128, 63488)
    (free host reshape) and every DMA slab z[:, c0:c1] is contiguous per
    partition (2-8KB descriptor runs, at line rate).
  - All Z traffic via big slabs on the SP HWDGE ring; the whole core's Z
    (62KB/partition) plus squares (62KB) stay resident in SBUF -- no ring
    buffers, minimal DMA count.
  - Squares fp8->fp8 split across ACT (square), DVE (tensor_mul) and Pool
    (gpsimd tensor_mul) by a static greedy schedule against estimated DMA
    arrival times; the DMA-tail tiles are reserved for the faster engines.
  - PE reduces with the per-pair R block [128, 2, 16] stationary in fp8
    DoubleRow perf mode: each matmul contracts 256 rows (two tiles) while
    streaming 512 PSUM columns at 0.5 cyc/col; 64 matmuls total into 4
    PSUM banks.  Pair 15 = tiles (29, 30), where the repeat appearance of
    tile 29 gets R=0 on host (keeps every matmul uniformly DoubleRow).
  - Host extracts the q'==q diagonal blocks of the [16, 2048] output and
    applies 1 - exp(-exp(s)) in f64.
"""

import sys

sys.path.insert(0, "/opt/trn_rl_repo")


# The agent image lacks antenv.axon_hooks; recreate it so trace=True works
# (bass_utils imports it lazily for NTFF profiling under axon).
def _install_ntff_hook_shim():
    import types
    if "antenv.axon_hooks" in sys.modules:
        return
    mod = types.ModuleType("antenv.axon_hooks")
    state = {"hook": None}
    mod.set_axon_ntff_profile_hook = lambda h: state.__setitem__("hook", h)
    mod.get_axon_ntff_profile_hook = lambda: state["hook"]
    sys.modules["antenv.axon_hooks"] = mod
    try:
        sys.path.insert(0, "/root/.axon_site")
        from trn_agent_boot.trn_boot import _ntff_profile_via_ctypes
        state["hook"] = _ntff_profile_via_ctypes("/opt/axon/libaxon_pjrt.so")
    except Exception:
        pass


_install_ntff_hook_shim()

import numpy as np

import concourse.bass as bass
import concourse.bacc as bacc
import concourse.mybir as mybir
from concourse.tile import TileContext
from concourse.bass_utils import run_bass_kernel_spmd

P = 128          # SBUF partitions
D = 128          # row length (feature dim)
Q = 16           # rows per partition per tile
T = 31           # tiles per core
FREE = Q * D     # 2048 free elems per partition per tile
ROWS_PER_PART = T * Q         # 496
NC_ROWS = P * ROWS_PER_PART   # 63488 rows per core
N_CORES = 8
N_FULL = 500000
MM_N = 512       # matmul moving-operand slice (PSUM: <=512 f32 out per bank)
NSLICES = FREE // MM_N        # 4
NPAIRS = 16
# pair k < 15 contracts tiles (2k, 2k+1); pair 15 contracts (29, 30) with the
# repeated tile 29 given R=0 on host.
PAIR_STARTS = [2 * k for k in range(15)] + [29]

Z_DT = mybir.dt.float8e4
R_DT = mybir.dt.float8e4
S_DT = mybir.dt.float8e4
Z_SCALE_IN = 512.0            # host multiplies Z by this before the fp8 cast

SLAB_SIZES = [1, 1, 2, 2, 3, 3, 3, 4, 4, 4, 2, 1, 1]   # tiles per DMA, sum=31
assert sum(SLAB_SIZES) == T


def _square_schedule():
    """Greedy tile->engine assignment using estimated DMA arrival times (us)
    and per-tile square costs; Pool is barred from the DMA-tail tiles."""
    cost = {"act": 1.80, "dve": 2.20, "pool": 4.30}
    arr = []
    tdone = 1.8
    for sz in SLAB_SIZES:
        tdone += 0.73 * sz
        arr += [tdone + 0.9] * sz
    free = {"act": 0.0, "dve": 0.0, "pool": 0.0}
    assign = []
    for t in range(T):
        cands = ("act", "dve", "pool") if t < T - 5 else ("act", "dve")
        e = min(cands, key=lambda e: (max(arr[t], free[e]) + cost[e], cost[e]))
        assign.append(e)
        free[e] = max(arr[t], free[e]) + cost[e]
    return assign


SQ_ENGINE = _square_schedule()

_cache = {}


def _np_dt(dt):
    return mybir.dt.np(dt)


def _build():
    nc = bacc.Bacc(trn_type="TRN2")
    z = nc.declare_dram_parameter("z", [P, T * FREE], Z_DT, isOutput=False)
    r = nc.declare_dram_parameter("r", [P, NPAIRS * 2 * Q], R_DT, isOutput=False)
    out = nc.declare_dram_parameter("out", [Q, FREE], mybir.dt.float32, isOutput=True)

    with TileContext(nc) as tc:
        with (
            tc.tile_pool(name="singles", bufs=1) as singles,
            tc.tile_pool(name="ppool", bufs=1, space="PSUM") as ppool,
        ):
            r_sb = singles.tile([P, NPAIRS, 2, Q], R_DT)
            nc.sync.dma_start(out=r_sb[:], in_=r[:])
            z_sb = singles.tile([P, T, FREE], Z_DT)
            sq_sb = singles.tile([P, T, FREE], S_DT)

            t0 = 0
            for sz in SLAB_SIZES:
                nc.sync.dma_start(
                    out=z_sb[:, t0:t0 + sz, :],
                    in_=z[:, t0 * FREE:(t0 + sz) * FREE],
                )
                t0 += sz

            for t in range(T):
                e = SQ_ENGINE[t]
                if e == "act":
                    nc.scalar.square(sq_sb[:, t, :], z_sb[:, t, :])
                elif e == "dve":
                    nc.vector.tensor_mul(sq_sb[:, t, :], z_sb[:, t, :], z_sb[:, t, :])
                else:
                    nc.gpsimd.tensor_mul(sq_sb[:, t, :], z_sb[:, t, :], z_sb[:, t, :])

            accs = [ppool.tile([Q, MM_N], mybir.dt.float32, name=f"acc{i}")
                    for i in range(NSLICES)]
            for k in range(NPAIRS):
                ks = PAIR_STARTS[k]
                for sl in range(NSLICES):
                    nc.tensor.matmul(
                        accs[sl][:],
                        r_sb[:, k, :, :],
                        sq_sb[:, ks:ks + 2, sl * MM_N:(sl + 1) * MM_N],
                        start=(k == 0),
                        stop=(k == NPAIRS - 1),
                        perf_mode=mybir.MatmulPerfMode.DoubleRow,
                    )

            out_sb = singles.tile([Q, FREE], mybir.dt.float32)
            for sl in range(NSLICES):
                copy_eng = nc.vector.tensor_copy if sl % 2 == 0 else nc.scalar.copy
                copy_eng(out_sb[:, sl * MM_N:(sl + 1) * MM_N], accs[sl][:])
            nc.sync.dma_start(out=out[:], in_=out_sb[:])
    nc.compile()
    return nc


def _get_nc():
    if "nc" not in _cache:
        _cache["nc"] = _build()
    return _cache["nc"]


def _shard(Z, R):
    np_z = _np_dt(Z_DT)
    np_r = _np_dt(R_DT)
    ZP = np.zeros((N_CORES * NC_ROWS, D), dtype=np_z)
    ZP[:N_FULL] = (Z * np.float32(Z_SCALE_IN)).astype(np_z)
    ZW = ZP.reshape(N_CORES, P, T * FREE)

    RP = np.zeros((N_CORES * NC_ROWS,), dtype=np.float32)
    RP[:N_FULL] = R
    RV = RP.reshape(N_CORES, P, T, Q)
    RW = np.zeros((N_CORES, P, NPAIRS, 2, Q), dtype=np.float32)
    RW[:, :, :15] = RV[:, :, :30].reshape(N_CORES, P, 15, 2, Q)
    RW[:, :, 15, 0] = 0.0
    RW[:, :, 15, 1] = RV[:, :, 30]
    RW8 = np.ascontiguousarray(RW.astype(np_r)).reshape(N_CORES, P, NPAIRS * 2 * Q)
    return [{"z": ZW[kk], "r": RW8[kk]} for kk in range(N_CORES)]


def _combine(results):
    idx = np.arange(Q)
    s = 0.0
    for res in results:
        C = np.asarray(res["out"], dtype=np.float64).reshape(Q, Q, D)
        s += C[idx, idx, :].sum()
    s /= float(Z_SCALE_IN) ** 2
    lam = np.exp(s)
    logits = 1.0 - np.exp(-lam)
    return np.float32(logits)


def _run(Z, R, trace=False, tmpdir=None):
    nc = _get_nc()
    in_maps = _shard(Z, R)
    return run_bass_kernel_spmd(nc, in_maps, core_ids=list(range(N_CORES)),
                                trace=trace, tmpdir=tmpdir)


def kernel(Z, R):
    assert Z.shape == (N_FULL, D) and R.shape == (N_FULL,)
    out = _run(np.asarray(Z), np.asarray(R), trace=False)
    return _combine(out.results)
